# revision 69
# baseline (speedup 1.0000x reference)
"""Causal self-attention TRN2 Bass kernel (v2: transposed AV).

Problem (hardcoded): B=2, S=2048, D=1024, H=16, DK=64, fp32.
  Q = einsum('bsd,hdk->bhsk', x, Wq); K, V likewise
  scores = Q K^T / sqrt(DK), causal mask, softmax
  out = (softmax @ V) concat heads @ Wo^T

Sharding: 8 cores = 2 batches x 4 head-groups (4 heads each).  Each core
computes QKV projections for its 4 heads, attention, and the partial Wo
product for its 256 columns of the concat dim.  The host sums the 4
partials per batch (tensor-parallel all-reduce done at unshard time) and
transposes the (D, S) partial back to (S, D).

Per-core layout / algorithm (differences from the v1 baseline):
  - AV runs TRANSPOSED: O_sub[q,dk+1] = P_chunk^T^T... i.e. per 128-query
    subtile, matmul(lhsT=P^T chunk [s,128q], rhs=[V_chunk|1] [s,65]) accum
    over key chunks.  Each accumulation step streams only 65 columns
    (vs 512-de in natural orientation), cutting AV PE time in half.
    The ones column gives the softmax denominator per-query at col 64.
  - normalization: recip(denom col) on DVE, multiply O*1/den on Pool with
    a free-dim broadcast, then transpose back to concat^T layout with a
    single matmul against a 128x128 identity (streams 128 cols).  That
    matmul also lands the result in psum where a cheap 128-col copy
    produces concat^T[64h:64h+64, q] in bf16.
  - P (post-exp), V, concat, Wo and the output partials are bf16: bf16
    matmuls run 1 cycle/col at any free size (no fp32r >=256 rule), the
    causal mask needs no memset patch-ups, and output DMA bytes halve.
    Q/K/x/scores stay fp32r for precision.
  - causal masking: exp on the true [de:512] range; affine_select zero
    fill only on the 128-col diagonal block of each crossing chunk.
  - softmax uses a fixed shift exp(s - 12) (scores are O(1); exact
    softmax is shift-invariant so only the ratio matters).
  - head: x group 0 and wqk00/wqk10 are split into 512-col DMA pieces
    across the SP/Pool queues so the first projection matmul issues at
    ~2.9us (vs ~5us), and a dummy 1-col matmul + 1-elem exp at t~0 start
    the PE p-state ramp and the ACT table load before real work arrives.
  - schedule: windows (0,0),(1,0),(0,1),(1,1),(0,3),(1,3),(0,2),(1,2).
    Each window emits scores+exp+mask only; its AV units become the NEXT
    window's leading filler (normalize/transpose latency hides behind
    that window's scores).  The exp stream paces every window, so filler
    (deferred projections, V chunks, wo stages) is pulled between chunks
    against a cumulative ACT-minus-PE ns budget.  GPSIMD never touches
    PSUM (illegal on HW); psum-side copies go to DVE, or to ACT in the
    late windows where the exp stream has ended.  The last window (1,2)
    emits its own units in-window at a 1-chunk lag with 128-col wo(3)
    chunks chasing each retired query subtile, so only ~2us of matmuls,
    copies and DMAs trail the final exp.
"""

import numpy as np
import ml_dtypes

import concourse.bacc as bacc
import concourse.mybir as mybir
import concourse.tile as tile
from concourse.bass_utils import run_bass_kernel_spmd

B, S, D, H, DK = 2, 2048, 1024, 16, 64
NCORES = 8
GROUPS = 4  # head groups per batch
HL = 4  # heads per core
NPAIR = 2  # head pairs per core
DC = D // 128  # 8 contraction chunks
SC = S // 128  # 16 key chunks
QT = S // 512  # 4 query tiles
NEGC = -12.0  # softmax shift: weights = exp(score - 12) / sum

MM_DT = mybir.dt.float32r  # x / Wq / Wk / Wv / scores path
BF = mybir.dt.bfloat16     # P / V / concat / Wo / output path

_PROG = None

_SENT = object()


def _pull(gen, k=1):
    if gen is None:
        return
    for _ in range(k):
        if next(gen, _SENT) is _SENT:
            return


def _drain(gen):
    if gen is None:
        return
    for _ in gen:
        pass


def _chain(*gens):
    for g in gens:
        if g is not None:
            yield from g


def _emit(nc, xT, wqk, wv, wo, ident, outT):
    f32 = mybir.dt.float32
    AF = mybir.ActivationFunctionType
    Alu = mybir.AluOpType

    tc = nc._tc  # set by _build

    with (
        tc.tile_pool(name="big", bufs=2) as big,
        tc.tile_pool(name="wgt", bufs=1) as wgt,
        tc.tile_pool(name="nrm", bufs=6) as nrm,
        tc.tile_pool(name="stg", bufs=3) as stg,
        tc.tile_pool(name="ps_st", bufs=2, space="PSUM") as ps_st,
        tc.tile_pool(name="ps_av", bufs=1, space="PSUM") as ps_av,
        tc.tile_pool(name="ps_fl", bufs=2, space="PSUM") as ps_fl,
    ):
        # ---------- constants + warm-up ----------
        v_sb = wgt.tile([128, SC * HL * 65], BF, tag="v", name="v_sb")
        v_view = v_sb[:].rearrange("p (c h k) -> p c h k", c=SC, h=HL)
        nc.vector.memset(v_view[:, :, :, 64:65], 1.0)
        neg_c = wgt.tile([128, 1], f32, tag="negc", name="neg_c")
        nc.vector.memset(neg_c[:], NEGC)
        junk = wgt.tile([128, 8], f32, tag="junk", name="junk")
        nc.vector.memset(junk[:], 1.0)
        # start the ACT table load (exp) at t~0 instead of inside window 0
        nc.scalar.activation(junk[0:1, 4:5], junk[0:1, 0:1], AF.Exp)
        # start the PE p-state ramp: 1-col matmuls, ~0 cost in the model
        warm_ps = ps_fl.tile([128, 512], f32, tag="fl", name="warm_ps")
        nc.tensor.matmul(
            warm_ps[0:1, 0:1], junk[:, 0:1], junk[:, 1:2], start=True, stop=True
        )

        # ---------- input DMA ----------
        # x arrives pre-transposed, grouped by 512-query tile then d-chunk:
        # x_one[p, st*DC*512 + c*512 + s'] = x[b].T[c*128+p, st*512+s']
        x_one = big.tile([128, DC * S], MM_DT, tag="xbig", bufs=1, name="x_one")
        g = DC * 512  # 4096 cols per query-tile group

        def xpiece(q, a, n):
            q.dma_start(out=x_one[:, a : a + n], in_=xT[:, a : a + n])

        wqk_sb = {}
        for qk in range(2):
            for p in range(NPAIR):
                wqk_sb[qk, p] = wgt.tile(
                    [128, DC * 128], MM_DT, tag=f"wqk{qk}{p}", name=f"wqk{qk}{p}"
                )
        wv_sb = wgt.tile([128, DC * HL * DK], MM_DT, tag="wv", name="wv_sb")
        id_sb = wgt.tile([128, 128], BF, tag="id", name="id_sb")
        wo_sb = []
        for p in range(NPAIR):
            wo_sb.append(wgt.tile([128, D], BF, tag=f"wo{p}", name=f"wo{p}"))

        # SP: x g0 even pieces + wqk00b, then the big group transfers
        xpiece(nc.sync, 0 * g + 0, 512)
        nc.sync.dma_start(out=wqk_sb[0, 0][:, 512:1024], in_=wqk[0, 0, :, 512:1024])
        xpiece(nc.sync, 0 * g + 1024, 512)
        xpiece(nc.sync, 0 * g + 2048, 1024)
        nc.sync.dma_start(out=wqk_sb[1, 0][:, 512:1024], in_=wqk[1, 0, :, 512:1024])
        nc.sync.dma_start(out=id_sb[:], in_=ident[:])
        xpiece(nc.sync, 1 * g, 2048)
        xpiece(nc.sync, 2 * g, 2048)
        xpiece(nc.sync, 3 * g, 2048)
        # Pool: wqk00a first (25ns seq -> earliest), odd x g0 pieces
        nc.gpsimd.dma_start(out=wqk_sb[0, 0][:, 0:512], in_=wqk[0, 0, :, 0:512])
        xpiece(nc.gpsimd, 0 * g + 512, 512)
        nc.gpsimd.dma_start(out=wqk_sb[1, 0][:, 0:512], in_=wqk[1, 0, :, 0:512])
        xpiece(nc.gpsimd, 0 * g + 1536, 512)
        xpiece(nc.gpsimd, 0 * g + 3072, 1024)
        xpiece(nc.gpsimd, 1 * g + 2048, 2048)
        xpiece(nc.gpsimd, 2 * g + 2048, 2048)
        xpiece(nc.gpsimd, 3 * g + 2048, 2048)
        for p in range(NPAIR):
            nc.gpsimd.dma_start(out=wo_sb[p][:], in_=wo[p])
        # ACT: remaining weights (exp doesn't start until ~11us)
        nc.scalar.dma_start(out=wv_sb[:, 0:1024], in_=wv[:, 0:1024])
        nc.scalar.dma_start(out=wv_sb[:, 1024:2048], in_=wv[:, 1024:2048])
        nc.scalar.dma_start(out=wqk_sb[0, 1][:], in_=wqk[0, 1])
        nc.scalar.dma_start(out=wqk_sb[1, 1][:], in_=wqk[1, 1])

        def xcol(c, s0, n):
            """x^T[c*128:(c+1)*128, s0:s0+n] — n must stay in one 512 tile."""
            st, s_ = divmod(s0, 512)
            assert s_ + n <= 512
            base = st * DC * 512 + c * 512 + s_
            return x_one[:, base : base + n]

        qt_sb = []
        kt_sb = []
        concat_sb = []
        for p in range(NPAIR):
            qt_sb.append(wgt.tile([128, S], MM_DT, tag=f"qt{p}", name=f"qt{p}"))
            kt_sb.append(wgt.tile([128, S], MM_DT, tag=f"kt{p}", name=f"kt{p}"))
            concat_sb.append(
                big.tile([128, S], BF, tag="big", name=f"concat{p}")
            )

        # ---------- filler generators (PE work in ~2-matmul quanta) ----------
        def prologue_qk0(p):
            # q/k st0 projections interleaved at 4-chunk granularity to
            # match the arrival order of the split head DMAs (q weights +
            # even x pieces land first).  Two psum groups stay open across
            # the interleave (separate banks, so the zero-region rule is
            # satisfied).
            psq = ps_fl.tile([128, 512], f32, tag="fl", name="pro_q")
            psk = ps_fl.tile([128, 512], f32, tag="fl", name="pro_k")
            for cs, qk, ps in (
                (range(0, 4), 0, psq),
                (range(0, 4), 1, psk),
                (range(4, 8), 0, psq),
                (range(4, 8), 1, psk),
            ):
                w = wqk_sb[qk, p]
                for c in cs:
                    nc.tensor.matmul(
                        ps[:],
                        w[:, c * 128 : (c + 1) * 128],
                        xcol(c, 0, 512),
                        start=(c == 0),
                        stop=(c == DC - 1),
                    )
                if cs.stop == DC:
                    dest = qt_sb[p] if qk == 0 else kt_sb[p]
                    nc.vector.tensor_copy(dest[:, 0:512], ps[:])

        def gen_proj(p, items):
            # items: list of (qk, st); qk=0 -> Q^T pair tile, 1 -> K^T
            for qk, st in items:
                dest = qt_sb[p] if qk == 0 else kt_sb[p]
                w = wqk_sb[qk, p]
                ps = ps_fl.tile([128, 512], f32, tag="fl", name="proj_ps")
                for c in range(DC):
                    nc.tensor.matmul(
                        ps[:],
                        w[:, c * 128 : (c + 1) * 128],
                        xcol(c, st * 512, 512),
                        start=(c == 0),
                        stop=(c == DC - 1),
                    )
                    if c % 2 == 1:
                        yield 427
                nc.vector.tensor_copy(
                    dest[:, st * 512 : (st + 1) * 512], ps[:]
                )

        def gen_v_proj(scs):
            # V natural layout (bf16) for all 4 heads of one key chunk
            for sc in scs:
                ps = ps_fl.tile([128, HL * DK], f32, tag="fl", name="vproj_ps")
                for c in range(DC):
                    nc.tensor.matmul(
                        ps[:],
                        xcol(c, sc * 128, 128),
                        wv_sb[:, c * 256 : (c + 1) * 256],
                        start=(c == 0),
                        stop=(c == DC - 1),
                    )
                    if c % 4 == 3:
                        yield 427
                nc.vector.tensor_copy(
                    v_view[:, sc, :, 0:64],
                    ps[:].rearrange("p (h k) -> p h k", h=HL),
                )

        wo_dma_q = [nc.sync, nc.gpsimd]

        def psum_copy(i, dst, src, tail):
            # GPSIMD can't access PSUM; alternate DVE with ACT (idle in the
            # tail region) when allowed
            if tail and i % 2 == 1:
                nc.scalar.activation(dst, src, AF.Copy)
            else:
                nc.vector.tensor_copy(dst, src)

        def gen_wo(qt, cos=None, tail=False):
            # partial^T[:, qt] = sum over pairs of wo_pair^T @ concat_pair
            for co in (cos if cos is not None else range(DC)):
                ps = ps_fl.tile([128, 512], f32, tag="fl", name="wo_ps")
                for p in range(NPAIR):
                    nc.tensor.matmul(
                        ps[:],
                        wo_sb[p][:, co * 128 : (co + 1) * 128],
                        concat_sb[p][:, qt * 512 : (qt + 1) * 512],
                        start=(p == 0),
                        stop=(p == NPAIR - 1),
                    )
                ob = stg.tile([128, 512], BF, tag="ob", name="ob")
                psum_copy(co, ob[:], ps[:], tail)
                wo_dma_q[co % 2].dma_start(
                    out=outT[co][:, qt * 512 : (qt + 1) * 512], in_=ob[:]
                )
                yield 427

        wo_obs = {}

        def gen_wo_qs(qt, qs):
            # one wo stage at 128-col granularity for query subtile qs,
            # chasing the last window's retiring units.  Copies stage into
            # shared 256-col ob tiles; the DMA fires on odd qs so the
            # transfer runs at full rate (512B runs) with half the count.
            # The last pair runs after the final exp, so ACT joins the
            # DMA queues.
            queues = [nc.sync, nc.gpsimd] + ([nc.scalar] if qs == 3 else [])
            q0 = qt * 512 + qs * 128
            for co in range(DC):
                ps = ps_fl.tile([128, 512], f32, tag="fl", name="wo1_ps")
                for p in range(NPAIR):
                    nc.tensor.matmul(
                        ps[:, 0:128],
                        wo_sb[p][:, co * 128 : (co + 1) * 128],
                        concat_sb[p][:, q0 : q0 + 128],
                        start=(p == 0),
                        stop=(p == NPAIR - 1),
                    )
                if qs % 2 == 0:
                    wo_obs[co] = stg.tile(
                        [128, 256], BF, tag="ob1", bufs=8, name="ob1"
                    )
                ob = wo_obs[co]
                half = qs % 2
                psum_copy(co, ob[:, half * 128 : half * 128 + 128],
                          ps[:, 0:128], True)
                if half == 1:
                    queues[co % len(queues)].dma_start(
                        out=outT[co][:, q0 - 128 : q0 + 128], in_=ob[:]
                    )
                yield 107

        # One persistent 2-bank psum tile for the AV units: half h of each
        # unit accumulates in bank h (cols h*512+0:65, group started/stopped
        # per unit, sequentially — the zero-region rule allows that) and the
        # [64,128] identity-transpose output reuses cols h*512+80:208 after
        # the group stops.  Slice-level WAR tracking then lets unit qs+1's
        # AVs start as soon as unit qs's recip+mul have read the bank — not
        # its ct copy (disjoint columns), which a pool-ring alloc would
        # conservatively wait for.
        av2 = ps_av.tile([128, 1024], f32, tag="av", name="av2")

        # ---------- attention window ----------
        def gen_units(p, qt, pts, chase_wo=False, tail=False):
            """Transposed AV units for window (p, qt): per (query-subtile,
            half), a back-to-back psum accumulation burst (one unit per 2KB
            bank: the HW start flag zeroes a whole bank, so groups can't
            interleave within one), then normalize (recip + broadcast mul)
            and transpose back into concat^T via an identity matmul whose
            [64,128] output reuses spare columns of the unit's own bank.
            Yields between bursts; intended to run as the leading filler of
            the NEXT window so the chain latencies hide behind its scores."""
            chase = None
            for qs in range(4):
                ons = []
                for half in range(2):
                    if chase is not None:
                        for _ in range(4):
                            _pull(chase)
                    av = av2[:, half * 512 : half * 512 + 512]
                    for sc in range(4 * qt + qs + 1):
                        nc.tensor.matmul(
                            av[:, 0:65],
                            pts[sc][
                                :, half * 512 + qs * 128 : half * 512 + (qs + 1) * 128
                            ],
                            v_view[:, sc, 2 * p + half, :],
                            start=(sc == 0),
                            stop=(sc == 4 * qt + qs),
                        )
                    rcol = nrm.tile([128, 1], f32, tag="rcol", name="rcol")
                    nc.vector.reciprocal_approx_fast(rcol[:], av[:, 64:65])
                    on = nrm.tile([128, 64], BF, tag="on", name="on")
                    # GPSIMD can't access PSUM on HW: normalize on DVE, or
                    # in the tail on the by-then-idle ACT (copy*scale)
                    if tail and half == 1 and qs == 3:
                        nc.scalar.activation(
                            on[:], av[:, 0:64], AF.Copy, scale=rcol[:]
                        )
                    else:
                        nc.vector.tensor_mul(
                            on[:], av[:, 0:64], rcol[:].broadcast_to([128, 64])
                        )
                    ons.append((av, on))
                    yield (4 * qt + qs + 1) * 27
                for half in range(2):
                    av, on = ons[half]
                    ct = av[0:64, 80:208]
                    nc.tensor.matmul(ct, on[:], id_sb[:], start=True, stop=True)
                    psum_copy(
                        half,
                        concat_sb[p][
                            64 * half : 64 * half + 64,
                            qt * 512 + qs * 128 : qt * 512 + (qs + 1) * 128,
                        ],
                        ct,
                        tail and qs == 3,
                    )
                    yield 53
                if chase_wo:
                    if chase is not None:
                        yield from chase
                    chase = gen_wo_qs(qt, qs)
            if chase is not None:
                yield from chase

        def attn_t(p, qt, must=None, spill=None, pulls=1, last=False,
                   tail=False):
            """scores + exp + causal mask for one (pair, query-tile) window,
            pulling `must` then `spill` between chunks (k quanta per chunk).
            `must` is fully drained before returning (later windows depend
            on it); leftover `spill` (order-independent wo work) is handed
            to the next window.  Returns a generator of [this window's AV
            units, then leftover spill].  With last=True the units run
            inside the window at a 1-chunk lag, qt=1 wo chunks chase them,
            and everything drains before returning."""
            nvalid = 4 * (qt + 1)

            fillers = [g for g in (must, spill) if g is not None]
            stats = {"pulled": 0, "budget": 0}
            carry = [0.0]

            def pull_fillers(ns_budget):
                # cumulative: overshoot from quantum granularity carries as
                # negative budget for the next chunk (overshoot delays the
                # next scores pair, stretching the ACT-paced window)
                want = ns_budget + carry[0]
                got = 0
                stats["budget"] += ns_budget
                while fillers and got < want:
                    r = next(fillers[0], _SENT)
                    if r is _SENT:
                        fillers.pop(0)
                    else:
                        got += r if r else 427
                carry[0] = want - got if fillers else 0.0
                stats["pulled"] += got

            pts = {}
            units = gen_units(p, qt, pts, chase_wo=True, tail=True) if last else None
            for sc in range(nvalid):
                de = max(0, sc * 128 - qt * 512)  # true diagonal offset
                d = min(de, 256)  # fp32r: keep scores free dim >= 256
                q0 = qt * 512 + d
                stp = ps_st.tile([128, 1024], f32, tag="st", name="st_ps")
                nc.tensor.matmul(
                    stp[:, d:512],
                    kt_sb[p][0:64, sc * 128 : (sc + 1) * 128],
                    qt_sb[p][0:64, q0 : (qt + 1) * 512],
                    start=True,
                    stop=True,
                    tile_position=(0, 0),
                )
                nc.tensor.matmul(
                    stp[:, 512 + d : 1024],
                    kt_sb[p][64:128, sc * 128 : (sc + 1) * 128],
                    qt_sb[p][64:128, q0 : (qt + 1) * 512],
                    start=True,
                    stop=True,
                    tile_position=(64, 0),
                )
                ptile = big.tile([128, 1024], BF, tag="pt", bufs=22, name="pt")
                pts[sc] = ptile
                if de == 0:
                    nc.scalar.activation(ptile[:], stp[:], AF.Exp, bias=neg_c[:])
                else:
                    # both head-halves in one fused 3-D AP, true range only
                    pt3 = ptile[:].rearrange("q (j n) -> q j n", j=2)[:, :, de:512]
                    st3 = stp[:].rearrange("q (j n) -> q j n", j=2)[:, :, de:512]
                    nc.scalar.activation(pt3, st3, AF.Exp, bias=neg_c[:])
                if sc >= 4 * qt:
                    # diagonal 128-col block: zero where q_loc < key part
                    blk = ptile[:].rearrange("q (j n) -> q j n", j=2)[
                        :, :, de : de + 128
                    ]
                    nc.gpsimd.affine_select(
                        blk,
                        blk,
                        pattern=[[0, 2], [1, 128]],
                        base=0,
                        channel_multiplier=-1,
                        compare_op=Alu.is_ge,
                        fill=0.0,
                    )
                act_ns = 2 * (512 - de) * 0.833 + 650
                pe_ns = 2 * (512 - d) * 0.4167
                pull_fillers(max(0.0, act_ns - pe_ns) * pulls)
                if last and sc > 4 * qt and not fillers:
                    # a unit per chunk becomes eligible (its last exp was
                    # issued the previous chunk); one unit = 4 yields (the
                    # wo chase chunks are emitted as side effects).  Gated
                    # on the fillers being drained: the chase chunks read
                    # concat columns the must-filler writes.
                    _pull(units, 4)
            import os as _os
            if _os.environ.get("ATTN_DEBUG"):
                print(f"window ({p},{qt}): budget {stats['budget']:.0f} "
                      f"pulled {stats['pulled']:.0f}")
            if last:
                _drain(units)
                for f in fillers:
                    _drain(f)
                return None
            if must is not None:
                _drain(must)
            return _chain(gen_units(p, qt, pts, tail=tail), spill)

        # ---------- schedule ----------
        # Windows in ascending qt order: the big qt=3 windows at the end
        # have the most ACT-paced slack, which is exactly when the wo
        # stages and deferred projections become available.  Each window's
        # AV units run as the next window's leading filler; the last
        # window (1,3) emits its own units in-window with 128-col wo(3)
        # chunks chasing each retired query subtile.
        prologue_qk0(0)
        u = attn_t(
            0, 0,
            must=_chain(gen_proj(1, [(0, 0), (1, 0)]), gen_v_proj(range(0, 4))),
            pulls=1.15,
        )
        u = attn_t(
            1, 0,
            must=_chain(u, gen_proj(0, [(0, 1), (1, 1)])),
            pulls=1.15,
        )
        u = attn_t(
            0, 1,
            must=_chain(u, gen_proj(1, [(0, 1), (1, 1)]),
                        gen_v_proj(range(4, 8))),
            pulls=1.15,
        )
        u = attn_t(
            1, 1,
            must=_chain(u, gen_proj(0, [(1, 2), (0, 2)])),
            pulls=1.15,
        )
        u = attn_t(
            0, 2,
            must=_chain(u, gen_proj(1, [(1, 2), (0, 2)]),
                        gen_v_proj(range(8, 12))),
            spill=gen_wo(1, range(0, 4), tail=True),
            pulls=0.95,
        )
        u = attn_t(
            1, 2,
            must=_chain(u, gen_proj(0, [(1, 3), (0, 3)])),
            spill=gen_wo(0, range(0, 4)),
            pulls=0.95,
        )
        u = attn_t(
            0, 3,
            must=_chain(u, gen_proj(1, [(1, 3), (0, 3)]),
                        gen_v_proj(range(12, 16))),
            spill=gen_wo(2, tail=True),
            pulls=0.95,
            tail=True,
        )
        attn_t(
            1, 3,
            must=_chain(u, gen_wo(0, range(6, 8), tail=True),
                        gen_wo(1, range(4, 8), tail=True),
                        gen_wo(0, range(6, 8), tail=True)),
            pulls=6,
            last=True,
        )


# revision 70
# speedup vs baseline: 1.0030x; 1.0030x over previous
"""Causal self-attention TRN2 Bass kernel (v2: transposed AV).

Problem (hardcoded): B=2, S=2048, D=1024, H=16, DK=64, fp32.
  Q = einsum('bsd,hdk->bhsk', x, Wq); K, V likewise
  scores = Q K^T / sqrt(DK), causal mask, softmax
  out = (softmax @ V) concat heads @ Wo^T

Sharding: 8 cores = 2 batches x 4 head-groups (4 heads each).  Each core
computes QKV projections for its 4 heads, attention, and the partial Wo
product for its 256 columns of the concat dim.  The host sums the 4
partials per batch (tensor-parallel all-reduce done at unshard time) and
transposes the (D, S) partial back to (S, D).

Per-core layout / algorithm (differences from the v1 baseline):
  - AV runs TRANSPOSED: O_sub[q,dk+1] = P_chunk^T^T... i.e. per 128-query
    subtile, matmul(lhsT=P^T chunk [s,128q], rhs=[V_chunk|1] [s,65]) accum
    over key chunks.  Each accumulation step streams only 65 columns
    (vs 512-de in natural orientation), cutting AV PE time in half.
    The ones column gives the softmax denominator per-query at col 64.
  - normalization: recip(denom col) on DVE, multiply O*1/den on Pool with
    a free-dim broadcast, then transpose back to concat^T layout with a
    single matmul against a 128x128 identity (streams 128 cols).  That
    matmul also lands the result in psum where a cheap 128-col copy
    produces concat^T[64h:64h+64, q] in bf16.
  - P (post-exp), V, concat, Wo and the output partials are bf16: bf16
    matmuls run 1 cycle/col at any free size (no fp32r >=256 rule), the
    causal mask needs no memset patch-ups, and output DMA bytes halve.
    Q/K/x/scores stay fp32r for precision.
  - causal masking: exp on the true [de:512] range; affine_select zero
    fill only on the 128-col diagonal block of each crossing chunk.
  - softmax uses a fixed shift exp(s - 12) (scores are O(1); exact
    softmax is shift-invariant so only the ratio matters).
  - head: x group 0 and wqk00/wqk10 are split into 512-col DMA pieces
    across the SP/Pool queues so the first projection matmul issues at
    ~2.9us (vs ~5us), and a dummy 1-col matmul + 1-elem exp at t~0 start
    the PE p-state ramp and the ACT table load before real work arrives.
  - schedule: windows (0,0),(1,0),(0,1),(1,1),(0,3),(1,3),(0,2),(1,2).
    Each window emits scores+exp+mask only; its AV units become the NEXT
    window's leading filler (normalize/transpose latency hides behind
    that window's scores).  The exp stream paces every window, so filler
    (deferred projections, V chunks, wo stages) is pulled between chunks
    against a cumulative ACT-minus-PE ns budget.  GPSIMD never touches
    PSUM (illegal on HW); psum-side copies go to DVE, or to ACT in the
    late windows where the exp stream has ended.  The last window (1,2)
    emits its own units in-window at a 1-chunk lag with 128-col wo(3)
    chunks chasing each retired query subtile, so only ~2us of matmuls,
    copies and DMAs trail the final exp.
"""

import numpy as np
import ml_dtypes

import concourse.bacc as bacc
import concourse.mybir as mybir
import concourse.tile as tile
from concourse.bass_utils import run_bass_kernel_spmd

B, S, D, H, DK = 2, 2048, 1024, 16, 64
NCORES = 8
GROUPS = 4  # head groups per batch
HL = 4  # heads per core
NPAIR = 2  # head pairs per core
DC = D // 128  # 8 contraction chunks
SC = S // 128  # 16 key chunks
QT = S // 512  # 4 query tiles
NEGC = -12.0  # softmax shift: weights = exp(score - 12) / sum

MM_DT = mybir.dt.float32r  # x / Wq / Wk / Wv / scores path
BF = mybir.dt.bfloat16     # P / V / concat / Wo / output path

_PROG = None

_SENT = object()


def _pull(gen, k=1):
    if gen is None:
        return
    for _ in range(k):
        if next(gen, _SENT) is _SENT:
            return


def _drain(gen):
    if gen is None:
        return
    for _ in gen:
        pass


def _chain(*gens):
    for g in gens:
        if g is not None:
            yield from g


def _emit(nc, xT, wqk, wv, wo, ident, outT):
    f32 = mybir.dt.float32
    AF = mybir.ActivationFunctionType
    Alu = mybir.AluOpType

    tc = nc._tc  # set by _build

    with (
        tc.tile_pool(name="big", bufs=2) as big,
        tc.tile_pool(name="wgt", bufs=1) as wgt,
        tc.tile_pool(name="nrm", bufs=6) as nrm,
        tc.tile_pool(name="stg", bufs=3) as stg,
        tc.tile_pool(name="ps_st", bufs=2, space="PSUM") as ps_st,
        tc.tile_pool(name="ps_av", bufs=1, space="PSUM") as ps_av,
        tc.tile_pool(name="ps_fl", bufs=2, space="PSUM") as ps_fl,
    ):
        # ---------- constants + warm-up ----------
        v_sb = wgt.tile([128, SC * HL * 65], BF, tag="v", name="v_sb")
        v_view = v_sb[:].rearrange("p (c h k) -> p c h k", c=SC, h=HL)
        nc.vector.memset(v_view[:, :, :, 64:65], 1.0)
        neg_c = wgt.tile([128, 1], f32, tag="negc", name="neg_c")
        nc.vector.memset(neg_c[:], NEGC)
        junk = wgt.tile([128, 8], f32, tag="junk", name="junk")
        nc.vector.memset(junk[:], 1.0)
        # start the ACT table load (exp) at t~0 instead of inside window 0
        nc.scalar.activation(junk[0:1, 4:5], junk[0:1, 0:1], AF.Exp)
        # start the PE p-state ramp: 1-col matmuls, ~0 cost in the model
        warm_ps = ps_fl.tile([128, 512], f32, tag="fl", name="warm_ps")
        nc.tensor.matmul(
            warm_ps[0:1, 0:1], junk[:, 0:1], junk[:, 1:2], start=True, stop=True
        )

        # ---------- input DMA ----------
        # x arrives pre-transposed, grouped by 512-query tile then d-chunk:
        # x_one[p, st*DC*512 + c*512 + s'] = x[b].T[c*128+p, st*512+s']
        x_one = big.tile([128, DC * S], MM_DT, tag="xbig", bufs=1, name="x_one")
        g = DC * 512  # 4096 cols per query-tile group

        def xpiece(q, a, n):
            q.dma_start(out=x_one[:, a : a + n], in_=xT[:, a : a + n])

        wqk_sb = {}
        for qk in range(2):
            for p in range(NPAIR):
                wqk_sb[qk, p] = wgt.tile(
                    [128, DC * 128], MM_DT, tag=f"wqk{qk}{p}", name=f"wqk{qk}{p}"
                )
        wv_sb = wgt.tile([128, DC * HL * DK], MM_DT, tag="wv", name="wv_sb")
        id_sb = wgt.tile([128, 128], BF, tag="id", name="id_sb")
        wo_sb = []
        for p in range(NPAIR):
            wo_sb.append(wgt.tile([128, D], BF, tag=f"wo{p}", name=f"wo{p}"))

        # SP: x g0 even pieces + wqk00b, then the big group transfers
        xpiece(nc.sync, 0 * g + 0, 512)
        nc.sync.dma_start(out=wqk_sb[0, 0][:, 512:1024], in_=wqk[0, 0, :, 512:1024])
        xpiece(nc.sync, 0 * g + 1024, 512)
        xpiece(nc.sync, 0 * g + 2048, 1024)
        nc.sync.dma_start(out=wqk_sb[1, 0][:, 512:1024], in_=wqk[1, 0, :, 512:1024])
        nc.sync.dma_start(out=id_sb[:], in_=ident[:])
        xpiece(nc.sync, 1 * g, 2048)
        xpiece(nc.sync, 2 * g, 2048)
        xpiece(nc.sync, 3 * g, 2048)
        # Pool: wqk00a first (25ns seq -> earliest), odd x g0 pieces
        nc.gpsimd.dma_start(out=wqk_sb[0, 0][:, 0:512], in_=wqk[0, 0, :, 0:512])
        xpiece(nc.gpsimd, 0 * g + 512, 512)
        nc.gpsimd.dma_start(out=wqk_sb[1, 0][:, 0:512], in_=wqk[1, 0, :, 0:512])
        xpiece(nc.gpsimd, 0 * g + 1536, 512)
        xpiece(nc.gpsimd, 0 * g + 3072, 1024)
        xpiece(nc.gpsimd, 1 * g + 2048, 2048)
        xpiece(nc.gpsimd, 2 * g + 2048, 2048)
        xpiece(nc.gpsimd, 3 * g + 2048, 2048)
        for p in range(NPAIR):
            nc.gpsimd.dma_start(out=wo_sb[p][:], in_=wo[p])
        # ACT: remaining weights (exp doesn't start until ~11us)
        nc.scalar.dma_start(out=wv_sb[:, 0:1024], in_=wv[:, 0:1024])
        nc.scalar.dma_start(out=wv_sb[:, 1024:2048], in_=wv[:, 1024:2048])
        nc.scalar.dma_start(out=wqk_sb[0, 1][:], in_=wqk[0, 1])
        nc.scalar.dma_start(out=wqk_sb[1, 1][:], in_=wqk[1, 1])

        def xcol(c, s0, n):
            """x^T[c*128:(c+1)*128, s0:s0+n] — n must stay in one 512 tile."""
            st, s_ = divmod(s0, 512)
            assert s_ + n <= 512
            base = st * DC * 512 + c * 512 + s_
            return x_one[:, base : base + n]

        qt_sb = []
        kt_sb = []
        concat_sb = []
        for p in range(NPAIR):
            qt_sb.append(wgt.tile([128, S], BF, tag=f"qt{p}", name=f"qt{p}"))
            kt_sb.append(wgt.tile([128, S], BF, tag=f"kt{p}", name=f"kt{p}"))
            concat_sb.append(
                big.tile([128, S], BF, tag="big", name=f"concat{p}")
            )

        # ---------- filler generators (PE work in ~2-matmul quanta) ----------
        def prologue_qk0(p):
            # q/k st0 projections interleaved at 4-chunk granularity to
            # match the arrival order of the split head DMAs (q weights +
            # even x pieces land first).  Two psum groups stay open across
            # the interleave (separate banks, so the zero-region rule is
            # satisfied).
            psq = ps_fl.tile([128, 512], f32, tag="fl", name="pro_q")
            psk = ps_fl.tile([128, 512], f32, tag="fl", name="pro_k")
            for cs, qk, ps in (
                (range(0, 4), 0, psq),
                (range(0, 4), 1, psk),
                (range(4, 8), 0, psq),
                (range(4, 8), 1, psk),
            ):
                w = wqk_sb[qk, p]
                for c in cs:
                    nc.tensor.matmul(
                        ps[:],
                        w[:, c * 128 : (c + 1) * 128],
                        xcol(c, 0, 512),
                        start=(c == 0),
                        stop=(c == DC - 1),
                    )
                if cs.stop == DC:
                    dest = qt_sb[p] if qk == 0 else kt_sb[p]
                    nc.vector.tensor_copy(dest[:, 0:512], ps[:])

        def gen_proj(p, items):
            # items: list of (qk, st); qk=0 -> Q^T pair tile, 1 -> K^T
            for qk, st in items:
                dest = qt_sb[p] if qk == 0 else kt_sb[p]
                w = wqk_sb[qk, p]
                ps = ps_fl.tile([128, 512], f32, tag="fl", name="proj_ps")
                for c in range(DC):
                    nc.tensor.matmul(
                        ps[:],
                        w[:, c * 128 : (c + 1) * 128],
                        xcol(c, st * 512, 512),
                        start=(c == 0),
                        stop=(c == DC - 1),
                    )
                    if c % 2 == 1:
                        yield 427
                nc.vector.tensor_copy(
                    dest[:, st * 512 : (st + 1) * 512], ps[:]
                )

        def gen_v_proj(scs):
            # V natural layout (bf16) for all 4 heads of one key chunk
            for sc in scs:
                ps = ps_fl.tile([128, HL * DK], f32, tag="fl", name="vproj_ps")
                for c in range(DC):
                    nc.tensor.matmul(
                        ps[:],
                        xcol(c, sc * 128, 128),
                        wv_sb[:, c * 256 : (c + 1) * 256],
                        start=(c == 0),
                        stop=(c == DC - 1),
                    )
                    if c % 4 == 3:
                        yield 427
                nc.vector.tensor_copy(
                    v_view[:, sc, :, 0:64],
                    ps[:].rearrange("p (h k) -> p h k", h=HL),
                )

        wo_dma_q = [nc.sync, nc.gpsimd]

        def psum_copy(i, dst, src, tail):
            # GPSIMD can't access PSUM; alternate DVE with ACT (idle in the
            # tail region) when allowed
            if tail and i % 2 == 1:
                nc.scalar.activation(dst, src, AF.Copy)
            else:
                nc.vector.tensor_copy(dst, src)

        def gen_wo(qt, cos=None, tail=False):
            # partial^T[:, qt] = sum over pairs of wo_pair^T @ concat_pair
            for co in (cos if cos is not None else range(DC)):
                ps = ps_fl.tile([128, 512], f32, tag="fl", name="wo_ps")
                for p in range(NPAIR):
                    nc.tensor.matmul(
                        ps[:],
                        wo_sb[p][:, co * 128 : (co + 1) * 128],
                        concat_sb[p][:, qt * 512 : (qt + 1) * 512],
                        start=(p == 0),
                        stop=(p == NPAIR - 1),
                    )
                ob = stg.tile([128, 512], BF, tag="ob", name="ob")
                psum_copy(co, ob[:], ps[:], tail)
                wo_dma_q[co % 2].dma_start(
                    out=outT[co][:, qt * 512 : (qt + 1) * 512], in_=ob[:]
                )
                yield 427

        wo_obs = {}

        def gen_wo_qs(qt, qs):
            # one wo stage at 128-col granularity for query subtile qs,
            # chasing the last window's retiring units.  Copies stage into
            # shared 256-col ob tiles; the DMA fires on odd qs so the
            # transfer runs at full rate (512B runs) with half the count.
            # The last pair runs after the final exp, so ACT joins the
            # DMA queues.
            queues = [nc.sync, nc.gpsimd] + ([nc.scalar] if qs == 3 else [])
            q0 = qt * 512 + qs * 128
            for co in range(DC):
                ps = ps_fl.tile([128, 512], f32, tag="fl", name="wo1_ps")
                for p in range(NPAIR):
                    nc.tensor.matmul(
                        ps[:, 0:128],
                        wo_sb[p][:, co * 128 : (co + 1) * 128],
                        concat_sb[p][:, q0 : q0 + 128],
                        start=(p == 0),
                        stop=(p == NPAIR - 1),
                    )
                if qs % 2 == 0:
                    wo_obs[co] = stg.tile(
                        [128, 256], BF, tag="ob1", bufs=8, name="ob1"
                    )
                ob = wo_obs[co]
                half = qs % 2
                psum_copy(co, ob[:, half * 128 : half * 128 + 128],
                          ps[:, 0:128], True)
                if half == 1:
                    queues[co % len(queues)].dma_start(
                        out=outT[co][:, q0 - 128 : q0 + 128], in_=ob[:]
                    )
                yield 107

        # One persistent 2-bank psum tile for the AV units: half h of each
        # unit accumulates in bank h (cols h*512+0:65, group started/stopped
        # per unit, sequentially — the zero-region rule allows that) and the
        # [64,128] identity-transpose output reuses cols h*512+80:208 after
        # the group stops.  Slice-level WAR tracking then lets unit qs+1's
        # AVs start as soon as unit qs's recip+mul have read the bank — not
        # its ct copy (disjoint columns), which a pool-ring alloc would
        # conservatively wait for.
        av2 = ps_av.tile([128, 1024], f32, tag="av", name="av2")

        # ---------- attention window ----------
        def gen_units(p, qt, pts, chase_wo=False, tail=False):
            """Transposed AV units for window (p, qt): per (query-subtile,
            half), a back-to-back psum accumulation burst (one unit per 2KB
            bank: the HW start flag zeroes a whole bank, so groups can't
            interleave within one), then normalize (recip + broadcast mul)
            and transpose back into concat^T via an identity matmul whose
            [64,128] output reuses spare columns of the unit's own bank.
            Yields between bursts; intended to run as the leading filler of
            the NEXT window so the chain latencies hide behind its scores."""
            chase = None
            for qs in range(4):
                ons = []
                for half in range(2):
                    if chase is not None:
                        for _ in range(4):
                            _pull(chase)
                    av = av2[:, half * 512 : half * 512 + 512]
                    for sc in range(4 * qt + qs + 1):
                        nc.tensor.matmul(
                            av[:, 0:65],
                            pts[sc][
                                :, half * 512 + qs * 128 : half * 512 + (qs + 1) * 128
                            ],
                            v_view[:, sc, 2 * p + half, :],
                            start=(sc == 0),
                            stop=(sc == 4 * qt + qs),
                        )
                    rcol = nrm.tile([128, 1], f32, tag="rcol", name="rcol")
                    nc.vector.reciprocal_approx_fast(rcol[:], av[:, 64:65])
                    on = nrm.tile([128, 64], BF, tag="on", name="on")
                    # GPSIMD can't access PSUM on HW: normalize on DVE, or
                    # in the tail on the by-then-idle ACT (copy*scale)
                    if tail and half == 1 and qs == 3:
                        nc.scalar.activation(
                            on[:], av[:, 0:64], AF.Copy, scale=rcol[:]
                        )
                    else:
                        nc.vector.tensor_mul(
                            on[:], av[:, 0:64], rcol[:].broadcast_to([128, 64])
                        )
                    ons.append((av, on))
                    yield (4 * qt + qs + 1) * 27
                for half in range(2):
                    av, on = ons[half]
                    ct = av[0:64, 80:208]
                    nc.tensor.matmul(ct, on[:], id_sb[:], start=True, stop=True)
                    psum_copy(
                        half,
                        concat_sb[p][
                            64 * half : 64 * half + 64,
                            qt * 512 + qs * 128 : qt * 512 + (qs + 1) * 128,
                        ],
                        ct,
                        tail and qs == 3,
                    )
                    yield 53
                if chase_wo:
                    if chase is not None:
                        yield from chase
                    chase = gen_wo_qs(qt, qs)
            if chase is not None:
                yield from chase

        def attn_t(p, qt, must=None, spill=None, pulls=1, last=False,
                   tail=False):
            """scores + exp + causal mask for one (pair, query-tile) window,
            pulling `must` then `spill` between chunks (k quanta per chunk).
            `must` is fully drained before returning (later windows depend
            on it); leftover `spill` (order-independent wo work) is handed
            to the next window.  Returns a generator of [this window's AV
            units, then leftover spill].  With last=True the units run
            inside the window at a 1-chunk lag, qt=1 wo chunks chase them,
            and everything drains before returning."""
            nvalid = 4 * (qt + 1)

            fillers = [g for g in (must, spill) if g is not None]
            stats = {"pulled": 0, "budget": 0}
            carry = [0.0]

            def pull_fillers(ns_budget):
                # cumulative: overshoot from quantum granularity carries as
                # negative budget for the next chunk (overshoot delays the
                # next scores pair, stretching the ACT-paced window)
                want = ns_budget + carry[0]
                got = 0
                stats["budget"] += ns_budget
                while fillers and got < want:
                    r = next(fillers[0], _SENT)
                    if r is _SENT:
                        fillers.pop(0)
                    else:
                        got += r if r else 427
                carry[0] = want - got if fillers else 0.0
                stats["pulled"] += got

            pts = {}
            units = gen_units(p, qt, pts, chase_wo=True, tail=True) if last else None
            for sc in range(nvalid):
                de = max(0, sc * 128 - qt * 512)  # true diagonal offset
                d = de  # bf16 scores: 1 cyc/col at any free size
                q0 = qt * 512 + d
                stp = ps_st.tile([128, 1024], f32, tag="st", name="st_ps")
                nc.tensor.matmul(
                    stp[:, d:512],
                    kt_sb[p][0:64, sc * 128 : (sc + 1) * 128],
                    qt_sb[p][0:64, q0 : (qt + 1) * 512],
                    start=True,
                    stop=True,
                    tile_position=(0, 0),
                )
                nc.tensor.matmul(
                    stp[:, 512 + d : 1024],
                    kt_sb[p][64:128, sc * 128 : (sc + 1) * 128],
                    qt_sb[p][64:128, q0 : (qt + 1) * 512],
                    start=True,
                    stop=True,
                    tile_position=(64, 0),
                )
                ptile = big.tile([128, 1024], BF, tag="pt", bufs=22, name="pt")
                pts[sc] = ptile
                if de == 0:
                    nc.scalar.activation(ptile[:], stp[:], AF.Exp, bias=neg_c[:])
                else:
                    # both head-halves in one fused 3-D AP, true range only
                    pt3 = ptile[:].rearrange("q (j n) -> q j n", j=2)[:, :, de:512]
                    st3 = stp[:].rearrange("q (j n) -> q j n", j=2)[:, :, de:512]
                    nc.scalar.activation(pt3, st3, AF.Exp, bias=neg_c[:])
                if sc >= 4 * qt:
                    # diagonal 128-col block: zero where q_loc < key part
                    blk = ptile[:].rearrange("q (j n) -> q j n", j=2)[
                        :, :, de : de + 128
                    ]
                    nc.gpsimd.affine_select(
                        blk,
                        blk,
                        pattern=[[0, 2], [1, 128]],
                        base=0,
                        channel_multiplier=-1,
                        compare_op=Alu.is_ge,
                        fill=0.0,
                    )
                act_ns = 2 * (512 - de) * 0.833 + 650
                pe_ns = 2 * (512 - d) * 0.4167
                pull_fillers(max(0.0, act_ns - pe_ns) * pulls)
                if last and sc > 4 * qt and not fillers:
                    # a unit per chunk becomes eligible (its last exp was
                    # issued the previous chunk); one unit = 4 yields (the
                    # wo chase chunks are emitted as side effects).  Gated
                    # on the fillers being drained: the chase chunks read
                    # concat columns the must-filler writes.
                    _pull(units, 4)
            import os as _os
            if _os.environ.get("ATTN_DEBUG"):
                print(f"window ({p},{qt}): budget {stats['budget']:.0f} "
                      f"pulled {stats['pulled']:.0f}")
            if last:
                _drain(units)
                for f in fillers:
                    _drain(f)
                return None
            if must is not None:
                _drain(must)
            return _chain(gen_units(p, qt, pts, tail=tail), spill)

        # ---------- schedule ----------
        # Windows in ascending qt order: the big qt=3 windows at the end
        # have the most ACT-paced slack, which is exactly when the wo
        # stages and deferred projections become available.  Each window's
        # AV units run as the next window's leading filler; the last
        # window (1,3) emits its own units in-window with 128-col wo(3)
        # chunks chasing each retired query subtile.
        prologue_qk0(0)
        u = attn_t(
            0, 0,
            must=_chain(gen_proj(1, [(0, 0), (1, 0)]), gen_v_proj(range(0, 4))),
            pulls=1.15,
        )
        u = attn_t(
            1, 0,
            must=_chain(u, gen_proj(0, [(0, 1), (1, 1)])),
            pulls=1.15,
        )
        u = attn_t(
            0, 1,
            must=_chain(u, gen_proj(1, [(0, 1), (1, 1)]),
                        gen_v_proj(range(4, 8))),
            pulls=1.15,
        )
        u = attn_t(
            1, 1,
            must=_chain(u, gen_proj(0, [(1, 2), (0, 2)])),
            pulls=1.15,
        )
        u = attn_t(
            0, 2,
            must=_chain(u, gen_proj(1, [(1, 2), (0, 2)]),
                        gen_v_proj(range(8, 12))),
            spill=gen_wo(1, range(0, 4), tail=True),
            pulls=0.95,
        )
        u = attn_t(
            1, 2,
            must=_chain(u, gen_proj(0, [(1, 3), (0, 3)])),
            spill=gen_wo(0, range(0, 4)),
            pulls=0.95,
        )
        u = attn_t(
            0, 3,
            must=_chain(u, gen_proj(1, [(1, 3), (0, 3)]),
                        gen_v_proj(range(12, 16))),
            spill=gen_wo(2, tail=True),
            pulls=0.95,
            tail=True,
        )
        attn_t(
            1, 3,
            must=_chain(u, gen_wo(0, range(6, 8), tail=True),
                        gen_wo(1, range(4, 8), tail=True),
                        gen_wo(0, range(6, 8), tail=True)),
            pulls=6,
            last=True,
        )
